# revision 54
# baseline (speedup 1.0000x reference)
"""DifferentialAttention (B=2, S=2048, D=2048, H=16, KVH=8) on 8 TRN2 NeuronCores.

Sharding: 8 cores = 2 (batch) x 4 (tensor-parallel head groups).
Core c = 4*b + r handles batch b and real heads 4r..4r+3:
  - column-parallel q/k/v projections (q heads 8r..8r+7, k heads 4r..4r+3,
    v heads 2r..2r+1), full causal differential attention for those heads,
  - row-parallel partial o_proj; host sums the 4 partials per batch.

Device math (per core):
  - Projections and o_proj run as fp8e4m3 DoubleRow matmuls with a 3-term
    hi/lo residual split (x_hi@W_hi + x_lo@W_hi + x_hi@W_lo); host pre-scales
    x/W into e4m3's normal range (2^1 / 2^6..2^9) and the descale rides the
    psum->sbuf copy (activation scale).  2x tensor throughput at ~0.2%
    element error.
  - scores computed transposed  S^T[k,q] = k . q  in f32r so exp -> AV needs
    no on-chip transposes; softmax without max subtraction.
  - causal masking: matmuls sliced to valid columns, last diagonal tile
    padded to 256 columns (f32r matmuls under 256 moving elements run at
    1/4 rate) and masked with a [0|tri] tile.
  - RMS-norm folded without reciprocal:  u = O1*R2 - lam*O2*R1  is the
    attention diff scaled per-column by R1*R2, which RMS-norm cancels;
    eps term becomes eps*(R1*R2)^2.  rsqrt = exp(-0.5*ln(x)); a manual
    LoadActFuncSet of the exp+ln table kills per-iteration table swaps.
  - engine balance: Pool (gpsimd) runs the RoPE rotate-half (shifted
    32-partition copies), causal masks, the R-derived epilogue chain, the
    variance partition_all_reduce, and the attn fp8-hi conversion; DVE
    runs RoPE mults, the O-derived chain, and phase-C bf16 combines; the
    scores S1/S2 of one k-tile share a [128,1024] psum so one exp covers
    both heads; phase-C partials leave as bf16 at 2^8 scale and the host
    gather descales (saves the on-chip scale op and half the out DMA).
"""

import math
import numpy as np

B, S, D = 2, 2048, 2048
H, KVH = 16, 8
Dh = 64
TP = 4
NCORES = 8
LAYER_IDX = 2
LAMBDA_INIT = 0.8 - 0.6 * math.exp(-0.3 * LAYER_IDX)
EPS = 1e-5
ROPE_THETA = 10000.0

# host-side fp8 range scaling (powers of two; descale folded on-device)
XS = 2.0 ** 1        # hidden_states
WQS = 2.0 ** 9       # Wq (includes Dh^-0.5)
WKS = 2.0 ** 6       # Wk
WVS = 2.0 ** 6       # Wv
WOS = 2.0 ** 7       # Wo (includes subln * (1-lambda_init))
ZS = 2.0             # attn (z) pre-o_proj, folded into sf via exp bias

_CACHE = {}


def _build_nc():
    import concourse.bass as bass  # noqa: F401
    import concourse.tile as tile
    from concourse import bacc, mybir

    from concourse import bass_isa

    F32 = mybir.dt.float32
    F32R = mybir.dt.float32r
    FP8 = mybir.dt.float8e4
    Act = mybir.ActivationFunctionType
    Alu = mybir.AluOpType
    DR = mybir.MatmulPerfMode.DoubleRow

    nc = bacc.Bacc("TRN2", target_bir_lowering=False, debug=False)

    xh_d = nc.dram_tensor("xh", [D, S], FP8, kind="ExternalInput")
    xl_d = nc.dram_tensor("xl", [D, S], FP8, kind="ExternalInput")
    wqh_d = nc.dram_tensor("wqh", [D, 512], FP8, kind="ExternalInput")
    wql_d = nc.dram_tensor("wql", [D, 512], FP8, kind="ExternalInput")
    # wk and wv packed side by side: 512B rows dodge the small-DMA penalty
    wkvh_d = nc.dram_tensor("wkvh", [D, 512], FP8, kind="ExternalInput")
    wkvl_d = nc.dram_tensor("wkvl", [D, 512], FP8, kind="ExternalInput")
    woh_d = nc.dram_tensor("woh", [512, D], FP8, kind="ExternalInput")
    wol_d = nc.dram_tensor("wol", [512, D], FP8, kind="ExternalInput")
    cosT_d = nc.dram_tensor("cosT", [128, S], F32, kind="ExternalInput")
    ssinT_d = nc.dram_tensor("ssinT", [128, S], F32, kind="ExternalInput")
    tri_d = nc.dram_tensor("tri", [128, 128], F32, kind="ExternalInput")
    m3_d = nc.dram_tensor("m3", [128, 256], F32, kind="ExternalInput")
    ones_d = nc.dram_tensor("ones", [128, 128], F32R, kind="ExternalInput")
    nlam_d = nc.dram_tensor("nlam", [128, 1], F32, kind="ExternalInput")
    # bf16 partial output at 2^8 scale; the host gather descales and upcasts
    BF16 = mybir.dt.bfloat16
    out_d = nc.dram_tensor("out", [S, D], BF16, kind="ExternalOutput")

    QDS = 2.0 ** -10     # q psum descale (XS*WQS)
    KDS = 2.0 ** -7      # k/v psum descale (XS*WKS)
    ODS = 2.0 ** -8      # o_proj psum descale (ZS*WOS)
    SEPS = math.sqrt(EPS)

    with tile.TileContext(nc) as tc:
        # exp+ln+copy share act-table 6 (natural_log_exp_and_others);
        # preloading it stops the table-load pass from thrashing 0<->5.
        nc.scalar.add_instruction(
            mybir.InstLoadActFuncSet(
                name=nc.get_next_instruction_name(),
                ins=[], outs=[], act_func_set_id=6,
            )
        )
        with tc.tile_pool(name="const", bufs=1) as constp, \
             tc.tile_pool(name="persist", bufs=1) as persist:

            tri = constp.tile([128, 128], F32, tag="tri")
            m3 = constp.tile([128, 256], F32, tag="m3")
            ones = constp.tile([128, 128], F32R, tag="ones")
            nlam = constp.tile([128, 1], F32, tag="nlam")

            qT_sb = [persist.tile([128, S], F32R, tag=f"qT{m}", name=f"qT{m}")
                     for m in range(4)]
            kTd = [persist.tile([128, S], F32R, tag=f"kTd{h}", name=f"kTd{h}")
                   for h in range(4)]
            v_sb = [persist.tile([128, 256], F32R, tag=f"v{ms}", name=f"v{ms}")
                    for ms in range(16)]
            # attn in fp8 hi/lo, kc-pair-interleaved for o_proj DoubleRow
            otf_h = [persist.tile([128, 2, S], FP8, tag=f"oh{pr}", name=f"oh{pr}")
                     for pr in range(2)]
            otf_l = [persist.tile([128, 2, S], FP8, tag=f"ol{pr}", name=f"ol{pr}")
                     for pr in range(2)]

            # ---------------- Phase A: projections + RoPE ----------------
            with tc.tile_pool(name="tabs", bufs=1) as tabs, \
                 tc.tile_pool(name="xtp", bufs=7) as xtp, \
                 tc.tile_pool(name="wres", bufs=1) as wres, \
                 tc.tile_pool(name="ropet", bufs=3) as rp, \
                 tc.tile_pool(name="psA", bufs=8, space="PSUM") as psA:

                cosT = tabs.tile([128, S], F32, tag="cos")
                ssinT = tabs.tile([128, S], F32, tag="ssin")

                def rope_core(ps, gc0, descale):
                    """RoPE a [128,512] psum tile; returns rotated sum parts."""
                    gsl = slice(gc0, gc0 + 512)
                    raw = rp.tile([128, 512], F32, tag="qraw", name="qraw")
                    nc.scalar.activation(raw[:], ps[:], Act.Copy, scale=descale)
                    mc = rp.tile([128, 512], F32, tag="mc", name="mc")
                    nc.vector.tensor_mul(mc[:], raw[:], cosT[:, gsl])
                    # rotate-half: partition-shifted 32-block copies on the
                    # otherwise idle Pool engine
                    rot = rp.tile([128, 512], F32, tag="rot", name="rot")
                    for blk in range(4):
                        d0 = blk * 32
                        s0 = (blk ^ 1) * 32
                        nc.gpsimd.tensor_copy(rot[d0:d0 + 32, :],
                                              raw[s0:s0 + 32, :])
                    ms_ = rp.tile([128, 512], F32, tag="ms", name="ms")
                    nc.vector.tensor_mul(ms_[:], rot[:], ssinT[:, gsl])
                    return mc, ms_

                def rope_epilogue_q(ps, m, gc0):
                    mc, ms_ = rope_core(ps, gc0, QDS)
                    nc.vector.tensor_add(qT_sb[m][:, gc0:gc0 + 512], mc[:], ms_[:])

                def rope_epilogue_kdup(ps, m, gc0):
                    gsl = slice(gc0, gc0 + 512)
                    mc, ms_ = rope_core(ps, gc0, KDS)
                    ktmp = rp.tile([128, 512], F32R, tag="ktmp", name="ktmp")
                    nc.vector.tensor_add(ktmp[:], mc[:], ms_[:])
                    for e in range(2):
                        src = ktmp[e * 64:e * 64 + 64, :]
                        nc.sync.dma_start(out=kTd[2 * m + e][0:64, gsl],
                                          in_=src)
                        nc.sync.dma_start(out=kTd[2 * m + e][64:128, gsl],
                                          in_=src)

                # resident fp8 weights [128, pairs, 2, n]
                wq_r = {}
                wkv_r = {}

                def load_w(nm, d, dst, width, split=False):
                    t = wres.tile([128, 8, 2, width], FP8, tag=nm, name=nm)
                    if split:  # two DMAs so the first matmuls start sooner
                        for g in range(2):
                            nc.sync.dma_start(
                                out=t[:, 4 * g:4 * g + 4, :, :],
                                in_=d[1024 * g:1024 * g + 1024, :]
                                    .rearrange("(pr j p) n -> p pr j n",
                                               pr=4, j=2, p=128))
                    else:
                        nc.sync.dma_start(
                            out=t[:],
                            in_=d[:].rearrange("(pr j p) n -> p pr j n",
                                               pr=8, j=2, p=128))
                    dst[nm[-1]] = t

                def load_rest():
                    # one-time tables ride the Activation DGE queue
                    nc.scalar.dma_start(out=cosT[:], in_=cosT_d[:])
                    nc.scalar.dma_start(out=ssinT[:], in_=ssinT_d[:])
                    load_w("wkvl", wkvl_d, wkv_r, 512)
                    nc.scalar.dma_start(out=tri[:], in_=tri_d[:])
                    nc.scalar.dma_start(out=m3[:], in_=m3_d[:])
                    nc.scalar.dma_start(out=ones[:], in_=ones_d[:])
                    nc.scalar.dma_start(out=nlam[:], in_=nlam_d[:])

                # (wq term, x term) order: lo operands arrive while the
                # earlier terms stream through the PE
                TERMS = (("h", "h"), ("h", "l"), ("l", "h"))

                for sh in range(4):
                    c0 = 512 * sh
                    # x^T fp8 hi/lo tiles: 2 groups of 8 contraction blocks
                    xt = {}

                    def load_xt(nm, d, g):
                        t = xtp.tile([128, 4, 2, 512], FP8, tag="xt",
                                     name=f"xt{nm}{g}")
                        nc.sync.dma_start(
                            out=t[:],
                            in_=d[g * 1024:g * 1024 + 1024, c0:c0 + 512]
                                .rearrange("(pr j p) n -> p pr j n",
                                           pr=4, j=2, p=128))
                        xt[nm, g] = t

                    # first-needed-first on the serial DMA device
                    load_xt("h", xh_d, 0)
                    if sh == 0:
                        load_w("wqh", wqh_d, wq_r, 512, split=True)
                    load_xt("h", xh_d, 1)
                    load_xt("l", xl_d, 0)
                    load_xt("l", xl_d, 1)
                    if sh == 0:
                        load_w("wql", wql_d, wq_r, 512)
                        load_w("wkvh", wkvh_d, wkv_r, 512)

                    def xt_rhs(nm, pr):
                        return xt[nm, pr // 4][:, pr % 4, :, :]

                    def do_q():
                        # q projection (fp8 DoubleRow, 8 pairs x 3 terms),
                        # term-major so the first matmuls need only hi parts
                        qps = {}
                        for m in range(4):
                            qps[m] = psA.tile([128, 512], F32, tag="pa",
                                              name=f"qps{m}")
                        for ti, (wn, xn) in enumerate(TERMS):
                            for pr in range(8):
                                for m in range(4):
                                    nc.tensor.matmul(
                                        qps[m][:],
                                        wq_r[wn][:, pr, :,
                                                 m * 128:m * 128 + 128],
                                        xt_rhs(xn, pr),
                                        start=(ti == 0 and pr == 0),
                                        stop=(ti == 2 and pr == 7),
                                        perf_mode=DR)
                        if sh == 0:
                            load_rest()
                        for m in range(4):
                            rope_epilogue_q(qps[m], m, c0)

                    def do_k():
                        # k projection (wkv cols 0:256 = wk: 2 head-blocks)
                        kps = {}
                        for m in range(2):
                            kps[m] = psA.tile([128, 512], F32, tag="pa",
                                              name=f"kps{m}")
                        for ti, (wn, xn) in enumerate(TERMS):
                            for pr in range(8):
                                for m in range(2):
                                    nc.tensor.matmul(
                                        kps[m][:],
                                        wkv_r[wn][:, pr, :,
                                                  m * 128:m * 128 + 128],
                                        xt_rhs(xn, pr),
                                        start=(ti == 0 and pr == 0),
                                        stop=(ti == 2 and pr == 7),
                                        perf_mode=DR)
                        for m in range(2):
                            rope_epilogue_kdup(kps[m], m, c0)

                    def do_v():
                        # v projection: v[s, j] (s on partitions; x is the
                        # stationary side; wkv cols 256:512 = wv)
                        vps = {}
                        for ms in range(4):
                            vps[ms] = psA.tile([128, 512], F32, tag="pa",
                                               name=f"vps{ms}")
                        for ti, (xn, wn) in enumerate(
                                (("h", "h"), ("l", "h"), ("h", "l"))):
                            for pr in range(8):
                                for ms in range(4):
                                    nc.tensor.matmul(
                                        vps[ms][:, 0:256],
                                        xt_rhs(xn, pr)[:, :,
                                                       ms * 128:ms * 128 + 128],
                                        wkv_r[wn][:, pr, :, 256:512],
                                        start=(ti == 0 and pr == 0),
                                        stop=(ti == 2 and pr == 7),
                                        perf_mode=DR)
                        for ms in range(4):
                            nc.scalar.activation(v_sb[sh * 4 + ms][:],
                                                 vps[ms][:, 0:256],
                                                 Act.Copy, scale=KDS)

                    # k first on later quarters: the k-rope -> kdup chain of
                    # the final quarter then hides under q/v matmuls, so
                    # phase B is not left waiting on kTd
                    if sh == 0:
                        do_q(); do_k(); do_v()
                    else:
                        do_k(); do_q(); do_v()

            # ---------------- Phases B+C ----------------
            with tc.tile_pool(name="wop", bufs=1) as wop:
              # prefetch o_proj weights during phase B
              wo_r = {}
              for nm, d in (("h", woh_d), ("l", wol_d)):
                  for n in range(4):
                      t = wop.tile([128, 2, 2, 512], FP8, tag=f"wo{nm}{n}",
                                   name=f"wo{nm}{n}")
                      nc.sync.dma_start(
                          out=t[:],
                          in_=d[:, n * 512:n * 512 + 512]
                              .rearrange("(pr j p) n -> p pr j n",
                                         pr=2, j=2, p=128))
                      wo_r[nm, n] = t

              # ---------------- Phase B: attention ----------------
              # qi-major so phase-C column chunks interleave after each qi
              # block, hiding the output DMA under phase-B compute
              with tc.tile_pool(name="etp", bufs=8) as etp, \
                 tc.tile_pool(name="ebp", bufs=16) as ebp, \
                 tc.tile_pool(name="psAcc", bufs=4, space="PSUM") as psAcc, \
                 tc.tile_pool(name="psS", bufs=2, space="PSUM") as psS:

                for qi, p in [(qq, pp) for qq in range(4) for pp in range(4)]:
                    vh = p // 2
                    q0 = 512 * qi
                    qsl = slice(q0, q0 + 512)
                    OT1 = psAcc.tile([128, 512], F32, tag="acc", name="OT1")
                    OT2 = psAcc.tile([128, 512], F32, tag="acc", name="OT2")
                    R1 = psAcc.tile([128, 512], F32, tag="acc", name="R1")
                    R2 = psAcc.tile([128, 512], F32, tag="acc", name="R2")

                    def avr(work, st, sp):
                        """AV + rowsum from an ET tile's two head halves."""
                        ki, osl, ET, s1sl, s2sl = work
                        vt = v_sb[ki][:, vh * 128:vh * 128 + 128]
                        nc.tensor.matmul(OT1[:, osl], vt, ET[:, s1sl],
                                         start=st, stop=sp)
                        nc.tensor.matmul(R1[:, osl], ones[:], ET[:, s1sl],
                                         start=st, stop=sp)
                        nc.tensor.matmul(OT2[:, osl], vt, ET[:, s2sl],
                                         start=st, stop=sp)
                        nc.tensor.matmul(R2[:, osl], ones[:], ET[:, s2sl],
                                         start=st, stop=sp)

                    # tiles: 4 diagonal (partial cols) then 4*qi full ones.
                    # S12 [128,1024] holds head1 scores in the low half and
                    # head2 in the high half; one exp covers both for full
                    # tiles.  AVR for tile t-1 is emitted after the S matmuls
                    # of tile t so the exp latency hides behind PE work.
                    prev = None
                    n_tiles = 4 + 4 * qi
                    for t in range(n_tiles):
                        diag = t < 4
                        ki = 4 * qi + t if diag else t - 4
                        vc = min(128 * t, 256) if diag else 0
                        ksl = slice(ki * 128, ki * 128 + 128)
                        S12 = psS.tile([128, 1024], F32, tag="s", name="S12")
                        nc.tensor.matmul(S12[:, vc:512], kTd[p][0:64, ksl],
                                         qT_sb[p][0:64, q0 + vc:q0 + 512],
                                         start=True, stop=True)
                        nc.tensor.matmul(S12[:, 512 + vc:1024],
                                         kTd[p][64:128, ksl],
                                         qT_sb[p][64:128, q0 + vc:q0 + 512],
                                         start=True, stop=True)
                        ET = etp.tile([128, 1024], F32R, tag="e", name="ET")
                        if diag:
                            nc.scalar.activation(ET[:, vc:512], S12[:, vc:512],
                                                 Act.Exp)
                            nc.scalar.activation(ET[:, 512 + vc:1024],
                                                 S12[:, 512 + vc:1024], Act.Exp)
                            if t < 3:
                                for b0 in (128 * t, 512 + 128 * t):
                                    nc.gpsimd.tensor_mul(ET[:, b0:b0 + 128],
                                                         ET[:, b0:b0 + 128],
                                                         tri[:])
                            else:
                                for b0 in (256, 768):
                                    nc.gpsimd.tensor_mul(ET[:, b0:b0 + 256],
                                                         ET[:, b0:b0 + 256],
                                                         m3[:])
                        else:
                            nc.scalar.activation(ET[:], S12[:], Act.Exp)
                        cur = (ki, slice(vc, 512), ET, slice(vc, 512),
                               slice(512 + vc, 1024))
                        if prev is not None:
                            avr(prev, t == 1, False)
                        prev = cur
                    avr(prev, n_tiles == 1, True)

                    # epilogue: u = O1*R2 - lam*O2*R1 (RMS-norm cancels R1*R2)
                    # engines read at most one PSUM operand per instruction,
                    # so the row sums are staged through SBUF first
                    r2s = ebp.tile([128, 512], F32, tag="eb", name="r2s")
                    nc.vector.tensor_copy(r2s[:], R2[:])
                    c1 = ebp.tile([128, 512], F32, tag="eb", name="c1")
                    nc.vector.tensor_mul(c1[:], OT1[:], r2s[:])
                    r1s = ebp.tile([128, 512], F32, tag="eb", name="r1s")
                    nc.scalar.copy(r1s[:], R1[:])
                    c2 = ebp.tile([128, 512], F32, tag="eb", name="c2")
                    nc.vector.tensor_mul(c2[:], OT2[:], r1s[:])
                    rr = ebp.tile([128, 512], F32, tag="eb", name="rr")
                    nc.gpsimd.tensor_mul(rr[:], r1s[:], r2s[:])
                    s1 = ebp.tile([128, 512], F32, tag="eb", name="s1")
                    # SEPS/ZS: scales pre by ZS^-2 so sf = ZS * rsqrt(pre)
                    nc.gpsimd.tensor_scalar_mul(s1[:], rr[:], SEPS / ZS)
                    u = ebp.tile([128, 512], F32R, tag="eb", name="u")
                    nc.vector.scalar_tensor_tensor(
                        u[:], c2[:], nlam[:, 0:1], c1[:], Alu.mult, Alu.add)
                    t2 = ebp.tile([128, 512], F32, tag="eb", name="t2")
                    nc.gpsimd.tensor_mul(t2[:], s1[:], s1[:])
                    sq = ebp.tile([128, 512], F32R, tag="eb", name="sq")
                    nc.vector.tensor_mul(sq[:], u[:], u[:])
                    # variance column-sum on the gpsimd partition reducer:
                    # no PE matmul, no psum slot, so the next iteration's
                    # accumulators are never blocked on this epilogue
                    varp = ebp.tile([128, 512], F32, tag="eb", name="varp")
                    nc.gpsimd.partition_all_reduce(varp[:], sq[:], 128,
                                                  bass_isa.ReduceOp.add)
                    pre = ebp.tile([128, 512], F32, tag="eb", name="pre")
                    nc.vector.scalar_tensor_tensor(
                        pre[:], varp[:], 1.0 / (128.0 * ZS * ZS), t2[:],
                        Alu.mult, Alu.add)
                    lnp = ebp.tile([128, 512], F32, tag="eb", name="lnp")
                    nc.scalar.activation(lnp[:], pre[:], Act.Ln)
                    sf = ebp.tile([128, 512], F32, tag="eb", name="sf")
                    nc.scalar.activation(sf[:], lnp[:], Act.Exp, scale=-0.5)
                    z = ebp.tile([128, 512], F32, tag="eb", name="z")
                    nc.vector.tensor_mul(z[:], u[:], sf[:])
                    oh = otf_h[p // 2][:, p % 2, qsl]
                    nc.gpsimd.tensor_copy(oh, z[:])
                    nc.vector.tensor_sub(otf_l[p // 2][:, p % 2, qsl], z[:], oh)

              # ---------------- Phase C: o_proj (fp8 DoubleRow partial) ---
              with tc.tile_pool(name="outp", bufs=3) as outp, \
                 tc.tile_pool(name="psC", bufs=6, space="PSUM") as psC:
                for m in range(16):
                    osb = outp.tile([128, 2048], BF16, tag="ob", name="osb")
                    msl = slice(m * 128, m * 128 + 128)
                    for n in range(4):
                        ps = psC.tile([128, 512], F32, tag="pc", name="pc")
                        first = True
                        for pr in range(2):
                            for an, wn in (("h", "h"), ("l", "h"),
                                           ("h", "l")):
                                at = (otf_h if an == "h" else otf_l)[pr]
                                nc.tensor.matmul(
                                    ps[:],
                                    at[:, :, msl],
                                    wo_r[wn, n][:, pr, :, :],
                                    start=first, stop=(pr == 1 and
                                                       an == "h" and
                                                       wn == "l"),
                                    perf_mode=DR)
                                first = False
                        nc.vector.tensor_copy(
                            osb[:, n * 512:n * 512 + 512], ps[:])
                        # per-n output DMA: overlaps the next n's matmuls
                        nc.sync.dma_start(
                            out=out_d[m * 128:m * 128 + 128,
                                      n * 512:n * 512 + 512],
                            in_=osb[:, n * 512:n * 512 + 512])

    nc.compile()
    return nc


def _host_tables():
    inv = ROPE_THETA ** (-np.arange(Dh, dtype=np.float64) / Dh)
    pos = np.arange(S, dtype=np.float64)
    fr = pos[:, None] * inv[None, :]              # [S, 64]
    cos = np.cos(fr).astype(np.float32)           # [S, 64]
    sin = np.sin(fr).astype(np.float32)
    d = np.arange(128) % 64
    cosT = cos[:, d].T.copy()                     # [128, S]
    sgn = np.where((np.arange(128) % 64) < 32, -1.0, 1.0).astype(np.float32)
    ssinT = (sin[:, d].T * sgn[:, None]).copy()
    tri = np.triu(np.ones((128, 128), np.float32))  # tri[k, q] = 1 if q >= k
    m3 = np.concatenate([np.zeros((128, 128), np.float32), tri], axis=1)
    ones = np.ones((128, 128), np.float32)
    return (np.ascontiguousarray(cosT), np.ascontiguousarray(ssinT), tri,
            np.ascontiguousarray(m3), ones)


def _fp8_split(a):
    import ml_dtypes
    E4 = ml_dtypes.float8_e4m3
    hi = np.ascontiguousarray(a).astype(E4)
    lo = (a - hi.astype(np.float32)).astype(E4)
    return hi, lo


def kernel(hidden_states, Wq, Wk, Wv, Wo,
           lambda_q1, lambda_k1, lambda_q2, lambda_k2, subln_weight):
    from concourse.bass_utils import run_bass_kernel_spmd

    if "nc" not in _CACHE:
        _CACHE["nc"] = _build_nc()
        _CACHE["tables"] = _host_tables()
    nc = _CACHE["nc"]
    cosT, ssinT, tri, m3, ones = _CACHE["tables"]

    f32 = np.float32
    hs = np.asarray(hidden_states, f32)
    Wq = np.asarray(Wq, f32)
    Wk = np.asarray(Wk, f32)
    Wv = np.asarray(Wv, f32)
    Wo = np.asarray(Wo, f32)
    subln = np.asarray(subln_weight, f32)

    lam1 = np.exp(np.sum(np.asarray(lambda_q1, f32) * np.asarray(lambda_k1, f32),
                         dtype=f32))
    lam2 = np.exp(np.sum(np.asarray(lambda_q2, f32) * np.asarray(lambda_k2, f32),
                         dtype=f32))
    lam_full = f32(lam1 - lam2 + LAMBDA_INIT)
    nlam_arr = np.full((128, 1), -lam_full, f32)

    scale = f32(Dh ** -0.5)
    wprime = (np.tile(subln, H) * f32(1.0 - LAMBDA_INIT)).astype(f32)  # [2048]
    WoS = Wo * wprime[None, :]

    in_maps = []
    for c in range(NCORES):
        b, r = c // TP, c % TP
        xh, xl = _fp8_split(hs[b].T * f32(XS))
        wqh, wql = _fp8_split((Wq[512 * r:512 * r + 512, :].T * f32(scale * WQS)))
        wkv = np.concatenate(
            [Wk[256 * r:256 * r + 256, :].T * f32(WKS),
             Wv[256 * r:256 * r + 256, :].T * f32(WVS)], axis=1)
        wkvh, wkvl = _fp8_split(wkv)
        woh, wol = _fp8_split(WoS[:, 512 * r:512 * r + 512].T * f32(WOS))
        in_maps.append({
            "xh": xh, "xl": xl,
            "wqh": wqh, "wql": wql, "wkvh": wkvh, "wkvl": wkvl,
            "woh": woh, "wol": wol,
            "cosT": cosT, "ssinT": ssinT, "tri": tri, "m3": m3, "ones": ones,
            "nlam": nlam_arr,
        })

    res = run_bass_kernel_spmd(nc, in_maps, core_ids=list(range(NCORES)))
    out = np.zeros((B, S, D), f32)
    ods = f32(1.0 / (ZS * WOS))
    for c in range(NCORES):
        out[c // TP] += res.results[c]["out"].astype(f32) * ods
    return out


# revision 60
# speedup vs baseline: 1.0094x; 1.0094x over previous
"""DifferentialAttention (B=2, S=2048, D=2048, H=16, KVH=8) on 8 TRN2 NeuronCores.

Sharding: 8 cores = 2 (batch) x 4 (tensor-parallel head groups).
Core c = 4*b + r handles batch b and real heads 4r..4r+3:
  - column-parallel q/k/v projections (q heads 8r..8r+7, k heads 4r..4r+3,
    v heads 2r..2r+1), full causal differential attention for those heads,
  - row-parallel partial o_proj; host sums the 4 partials per batch.

Device math (per core):
  - Projections and o_proj run as fp8e4m3 DoubleRow matmuls with a 3-term
    hi/lo residual split (x_hi@W_hi + x_lo@W_hi + x_hi@W_lo); host pre-scales
    x/W into e4m3's normal range (2^1 / 2^6..2^9) and the descale rides the
    psum->sbuf copy (activation scale).  2x tensor throughput at ~0.2%
    element error.
  - scores computed transposed  S^T[k,q] = k . q  in f32r so exp -> AV needs
    no on-chip transposes; softmax without max subtraction.
  - causal masking: matmuls sliced to valid columns, last diagonal tile
    padded to 256 columns (f32r matmuls under 256 moving elements run at
    1/4 rate) and masked with a [0|tri] tile.
  - RMS-norm folded without reciprocal:  u = O1*R2 - lam*O2*R1  is the
    attention diff scaled per-column by R1*R2, which RMS-norm cancels;
    eps term becomes eps*(R1*R2)^2.  rsqrt = exp(-0.5*ln(x)); a manual
    LoadActFuncSet of the exp+ln table kills per-iteration table swaps.
  - engine balance: Pool (gpsimd) runs the RoPE rotate-half (shifted
    32-partition copies), causal masks, the R-derived epilogue chain, the
    variance partition_all_reduce, and the attn fp8-hi conversion; DVE
    runs RoPE mults, the O-derived chain, and phase-C bf16 combines; the
    scores S1/S2 of one k-tile share a [128,1024] psum so one exp covers
    both heads; phase-C partials leave as bf16 at 2^8 scale and the host
    gather descales (saves the on-chip scale op and half the out DMA).
"""

import math
import numpy as np

B, S, D = 2, 2048, 2048
H, KVH = 16, 8
Dh = 64
TP = 4
NCORES = 8
LAYER_IDX = 2
LAMBDA_INIT = 0.8 - 0.6 * math.exp(-0.3 * LAYER_IDX)
EPS = 1e-5
ROPE_THETA = 10000.0

# host-side fp8 range scaling (powers of two; descale folded on-device)
XS = 2.0 ** 1        # hidden_states
WQS = 2.0 ** 9       # Wq (includes Dh^-0.5)
WKS = 2.0 ** 6       # Wk
WVS = 2.0 ** 6       # Wv
WOS = 2.0 ** 7       # Wo (includes subln * (1-lambda_init))
ZS = 2.0             # attn (z) pre-o_proj, folded into sf via exp bias

_CACHE = {}


def _build_nc():
    import concourse.bass as bass  # noqa: F401
    import concourse.tile as tile
    from concourse import bacc, mybir

    from concourse import bass_isa

    F32 = mybir.dt.float32
    F32R = mybir.dt.float32r
    FP8 = mybir.dt.float8e4
    Act = mybir.ActivationFunctionType
    Alu = mybir.AluOpType
    DR = mybir.MatmulPerfMode.DoubleRow

    nc = bacc.Bacc("TRN2", target_bir_lowering=False, debug=False)

    xh_d = nc.dram_tensor("xh", [D, S], FP8, kind="ExternalInput")
    xl_d = nc.dram_tensor("xl", [D, S], FP8, kind="ExternalInput")
    wqh_d = nc.dram_tensor("wqh", [D, 512], FP8, kind="ExternalInput")
    wql_d = nc.dram_tensor("wql", [D, 512], FP8, kind="ExternalInput")
    # wk and wv packed side by side: 512B rows dodge the small-DMA penalty
    wkvh_d = nc.dram_tensor("wkvh", [D, 512], FP8, kind="ExternalInput")
    wkvl_d = nc.dram_tensor("wkvl", [D, 512], FP8, kind="ExternalInput")
    woh_d = nc.dram_tensor("woh", [512, D], FP8, kind="ExternalInput")
    wol_d = nc.dram_tensor("wol", [512, D], FP8, kind="ExternalInput")
    cosT_d = nc.dram_tensor("cosT", [128, S], F32, kind="ExternalInput")
    ssinT_d = nc.dram_tensor("ssinT", [128, S], F32, kind="ExternalInput")
    tri_d = nc.dram_tensor("tri", [128, 128], F32, kind="ExternalInput")
    m3_d = nc.dram_tensor("m3", [128, 256], F32, kind="ExternalInput")
    ones_d = nc.dram_tensor("ones", [128, 128], F32R, kind="ExternalInput")
    nlam_d = nc.dram_tensor("nlam", [128, 1], F32, kind="ExternalInput")
    # bf16 partial output at 2^8 scale; the host gather descales and upcasts
    BF16 = mybir.dt.bfloat16
    out_d = nc.dram_tensor("out", [S, D], BF16, kind="ExternalOutput")

    QDS = 2.0 ** -10     # q psum descale (XS*WQS)
    KDS = 2.0 ** -7      # k/v psum descale (XS*WKS)
    ODS = 2.0 ** -8      # o_proj psum descale (ZS*WOS)
    SEPS = math.sqrt(EPS)

    with tile.TileContext(nc) as tc:
        # exp+ln+copy share act-table 6 (natural_log_exp_and_others);
        # preloading it stops the table-load pass from thrashing 0<->5.
        nc.scalar.add_instruction(
            mybir.InstLoadActFuncSet(
                name=nc.get_next_instruction_name(),
                ins=[], outs=[], act_func_set_id=6,
            )
        )
        with tc.tile_pool(name="const", bufs=1) as constp, \
             tc.tile_pool(name="persist", bufs=1) as persist:

            tri = constp.tile([128, 128], F32, tag="tri")
            m3 = constp.tile([128, 256], F32, tag="m3")
            ones = constp.tile([128, 128], F32R, tag="ones")
            nlam = constp.tile([128, 1], F32, tag="nlam")

            qT_sb = [persist.tile([128, S], F32R, tag=f"qT{m}", name=f"qT{m}")
                     for m in range(4)]
            kTd = [persist.tile([128, S], F32R, tag=f"kTd{h}", name=f"kTd{h}")
                   for h in range(4)]
            v_sb = [persist.tile([128, 256], F32R, tag=f"v{ms}", name=f"v{ms}")
                    for ms in range(16)]
            # attn in fp8 hi/lo, kc-pair-interleaved for o_proj DoubleRow
            otf_h = [persist.tile([128, 2, S], FP8, tag=f"oh{pr}", name=f"oh{pr}")
                     for pr in range(2)]
            otf_l = [persist.tile([128, 2, S], FP8, tag=f"ol{pr}", name=f"ol{pr}")
                     for pr in range(2)]

            # ---------------- Phase A: projections + RoPE ----------------
            with tc.tile_pool(name="tabs", bufs=1) as tabs, \
                 tc.tile_pool(name="xtp", bufs=7) as xtp, \
                 tc.tile_pool(name="wres", bufs=1) as wres, \
                 tc.tile_pool(name="ropet", bufs=3) as rp, \
                 tc.tile_pool(name="psA", bufs=8, space="PSUM") as psA:

                cosT = tabs.tile([128, S], F32, tag="cos")
                ssinT = tabs.tile([128, S], F32, tag="ssin")

                def rope_core(ps, gc0, descale):
                    """RoPE a [128,512] psum tile; returns rotated sum parts."""
                    gsl = slice(gc0, gc0 + 512)
                    raw = rp.tile([128, 512], F32, tag="qraw", name="qraw")
                    nc.scalar.activation(raw[:], ps[:], Act.Copy, scale=descale)
                    mc = rp.tile([128, 512], F32, tag="mc", name="mc")
                    nc.vector.tensor_mul(mc[:], raw[:], cosT[:, gsl])
                    # rotate-half: partition-shifted 32-block copies on the
                    # otherwise idle Pool engine
                    rot = rp.tile([128, 512], F32, tag="rot", name="rot")
                    for blk in range(4):
                        d0 = blk * 32
                        s0 = (blk ^ 1) * 32
                        nc.gpsimd.tensor_copy(rot[d0:d0 + 32, :],
                                              raw[s0:s0 + 32, :])
                    ms_ = rp.tile([128, 512], F32, tag="ms", name="ms")
                    nc.vector.tensor_mul(ms_[:], rot[:], ssinT[:, gsl])
                    return mc, ms_

                def rope_epilogue_q(ps, m, gc0):
                    mc, ms_ = rope_core(ps, gc0, QDS)
                    nc.vector.tensor_add(qT_sb[m][:, gc0:gc0 + 512], mc[:], ms_[:])

                def rope_epilogue_kdup(ps, m, gc0):
                    gsl = slice(gc0, gc0 + 512)
                    mc, ms_ = rope_core(ps, gc0, KDS)
                    ktmp = rp.tile([128, 512], F32R, tag="ktmp", name="ktmp")
                    nc.vector.tensor_add(ktmp[:], mc[:], ms_[:])
                    for e in range(2):
                        src = ktmp[e * 64:e * 64 + 64, :]
                        nc.sync.dma_start(out=kTd[2 * m + e][0:64, gsl],
                                          in_=src)
                        nc.sync.dma_start(out=kTd[2 * m + e][64:128, gsl],
                                          in_=src)

                # resident fp8 weights [128, pairs, 2, n]
                wq_r = {}
                wkv_r = {}

                def load_w(nm, d, dst, width, split=False):
                    t = wres.tile([128, 8, 2, width], FP8, tag=nm, name=nm)
                    if split:  # two DMAs so the first matmuls start sooner
                        for g in range(2):
                            nc.sync.dma_start(
                                out=t[:, 4 * g:4 * g + 4, :, :],
                                in_=d[1024 * g:1024 * g + 1024, :]
                                    .rearrange("(pr j p) n -> p pr j n",
                                               pr=4, j=2, p=128))
                    else:
                        nc.sync.dma_start(
                            out=t[:],
                            in_=d[:].rearrange("(pr j p) n -> p pr j n",
                                               pr=8, j=2, p=128))
                    dst[nm[-1]] = t

                def load_rest():
                    # one-time tables ride the Activation DGE queue
                    nc.scalar.dma_start(out=cosT[:], in_=cosT_d[:])
                    nc.scalar.dma_start(out=ssinT[:], in_=ssinT_d[:])
                    load_w("wkvl", wkvl_d, wkv_r, 512)
                    nc.scalar.dma_start(out=tri[:], in_=tri_d[:])
                    nc.scalar.dma_start(out=m3[:], in_=m3_d[:])
                    nc.scalar.dma_start(out=ones[:], in_=ones_d[:])
                    nc.scalar.dma_start(out=nlam[:], in_=nlam_d[:])

                # (wq term, x term) order: lo operands arrive while the
                # earlier terms stream through the PE
                TERMS = (("h", "h"), ("h", "l"), ("l", "h"))

                for sh in range(4):
                    c0 = 512 * sh
                    # x^T fp8 hi/lo tiles: 2 groups of 8 contraction blocks
                    xt = {}

                    def load_xt(nm, d, g):
                        t = xtp.tile([128, 4, 2, 512], FP8, tag="xt",
                                     name=f"xt{nm}{g}")
                        nc.sync.dma_start(
                            out=t[:],
                            in_=d[g * 1024:g * 1024 + 1024, c0:c0 + 512]
                                .rearrange("(pr j p) n -> p pr j n",
                                           pr=4, j=2, p=128))
                        xt[nm, g] = t

                    # first-needed-first on the serial DMA device
                    load_xt("h", xh_d, 0)
                    if sh == 0:
                        load_w("wqh", wqh_d, wq_r, 512, split=True)
                    load_xt("h", xh_d, 1)
                    load_xt("l", xl_d, 0)
                    load_xt("l", xl_d, 1)
                    if sh == 0:
                        load_w("wql", wql_d, wq_r, 512)
                        load_w("wkvh", wkvh_d, wkv_r, 512)

                    def xt_rhs(nm, pr):
                        return xt[nm, pr // 4][:, pr % 4, :, :]

                    def do_q():
                        # q projection (fp8 DoubleRow, 8 pairs x 3 terms),
                        # term-major so the first matmuls need only hi parts
                        qps = {}
                        for m in range(4):
                            qps[m] = psA.tile([128, 512], F32, tag="pa",
                                              name=f"qps{m}")
                        for ti, (wn, xn) in enumerate(TERMS):
                            for pr in range(8):
                                for m in range(4):
                                    nc.tensor.matmul(
                                        qps[m][:],
                                        wq_r[wn][:, pr, :,
                                                 m * 128:m * 128 + 128],
                                        xt_rhs(xn, pr),
                                        start=(ti == 0 and pr == 0),
                                        stop=(ti == 2 and pr == 7),
                                        perf_mode=DR)
                        if sh == 0:
                            load_rest()
                        for m in range(4):
                            rope_epilogue_q(qps[m], m, c0)

                    def do_k():
                        # k projection (wkv cols 0:256 = wk: 2 head-blocks)
                        kps = {}
                        for m in range(2):
                            kps[m] = psA.tile([128, 512], F32, tag="pa",
                                              name=f"kps{m}")
                        for ti, (wn, xn) in enumerate(TERMS):
                            for pr in range(8):
                                for m in range(2):
                                    nc.tensor.matmul(
                                        kps[m][:],
                                        wkv_r[wn][:, pr, :,
                                                  m * 128:m * 128 + 128],
                                        xt_rhs(xn, pr),
                                        start=(ti == 0 and pr == 0),
                                        stop=(ti == 2 and pr == 7),
                                        perf_mode=DR)
                        for m in range(2):
                            rope_epilogue_kdup(kps[m], m, c0)

                    def do_v():
                        # v projection: v[s, j] (s on partitions; x is the
                        # stationary side; wkv cols 256:512 = wv)
                        vps = {}
                        for ms in range(4):
                            vps[ms] = psA.tile([128, 512], F32, tag="pa",
                                               name=f"vps{ms}")
                        for ti, (xn, wn) in enumerate(
                                (("h", "h"), ("l", "h"), ("h", "l"))):
                            for pr in range(8):
                                for ms in range(4):
                                    nc.tensor.matmul(
                                        vps[ms][:, 0:256],
                                        xt_rhs(xn, pr)[:, :,
                                                       ms * 128:ms * 128 + 128],
                                        wkv_r[wn][:, pr, :, 256:512],
                                        start=(ti == 0 and pr == 0),
                                        stop=(ti == 2 and pr == 7),
                                        perf_mode=DR)
                        for ms in range(4):
                            nc.scalar.activation(v_sb[sh * 4 + ms][:],
                                                 vps[ms][:, 0:256],
                                                 Act.Copy, scale=KDS)

                    # k first on later quarters: the k-rope -> kdup chain of
                    # the final quarter then hides under q/v matmuls, so
                    # phase B is not left waiting on kTd
                    if sh == 0:
                        do_q(); do_k(); do_v()
                    else:
                        do_k(); do_q(); do_v()

            # ---------------- Phases B+C ----------------
            with tc.tile_pool(name="wop", bufs=1) as wop:
              # prefetch o_proj weights during phase B
              wo_r = {}
              for nm, d in (("h", woh_d), ("l", wol_d)):
                  for n in range(4):
                      t = wop.tile([128, 2, 2, 512], FP8, tag=f"wo{nm}{n}",
                                   name=f"wo{nm}{n}")
                      nc.sync.dma_start(
                          out=t[:],
                          in_=d[:, n * 512:n * 512 + 512]
                              .rearrange("(pr j p) n -> p pr j n",
                                         pr=2, j=2, p=128))
                      wo_r[nm, n] = t

              # ---------------- Phase B: attention ----------------
              # qi-major so phase-C column chunks interleave after each qi
              # block, hiding the output DMA under phase-B compute
              with tc.tile_pool(name="etp", bufs=8) as etp, \
                 tc.tile_pool(name="ebp", bufs=16) as ebp, \
                 tc.tile_pool(name="psAcc", bufs=4, space="PSUM") as psAcc, \
                 tc.tile_pool(name="psS", bufs=2, space="PSUM") as psS:

                for qi, p in [(qq, pp) for qq in range(4) for pp in range(4)]:
                    vh = p // 2
                    q0 = 512 * qi
                    qsl = slice(q0, q0 + 512)
                    OT1 = psAcc.tile([128, 512], F32, tag="acc", name="OT1")
                    OT2 = psAcc.tile([128, 512], F32, tag="acc", name="OT2")
                    R1 = psAcc.tile([128, 512], F32, tag="acc", name="R1")
                    R2 = psAcc.tile([128, 512], F32, tag="acc", name="R2")

                    def avr(work, st, sp):
                        """AV + rowsum from an ET tile's two head halves."""
                        ki, osl, ET, s1sl, s2sl = work
                        vt = v_sb[ki][:, vh * 128:vh * 128 + 128]
                        nc.tensor.matmul(OT1[:, osl], vt, ET[:, s1sl],
                                         start=st, stop=sp)
                        nc.tensor.matmul(R1[:, osl], ones[:], ET[:, s1sl],
                                         start=st, stop=sp)
                        nc.tensor.matmul(OT2[:, osl], vt, ET[:, s2sl],
                                         start=st, stop=sp)
                        nc.tensor.matmul(R2[:, osl], ones[:], ET[:, s2sl],
                                         start=st, stop=sp)

                    # tiles: 4 diagonal (partial cols) then 4*qi full ones.
                    # S12 [128,1024] holds head1 scores in the low half and
                    # head2 in the high half; one exp covers both for full
                    # tiles.  AVR for tile t-1 is emitted after the S matmuls
                    # of tile t so the exp latency hides behind PE work.
                    prev = None
                    n_tiles = 4 + 4 * qi
                    for t in range(n_tiles):
                        diag = t < 4
                        ki = 4 * qi + t if diag else t - 4
                        vc = min(128 * t, 256) if diag else 0
                        ksl = slice(ki * 128, ki * 128 + 128)
                        S12 = psS.tile([128, 1024], F32, tag="s", name="S12")
                        nc.tensor.matmul(S12[:, vc:512], kTd[p][0:64, ksl],
                                         qT_sb[p][0:64, q0 + vc:q0 + 512],
                                         start=True, stop=True)
                        nc.tensor.matmul(S12[:, 512 + vc:1024],
                                         kTd[p][64:128, ksl],
                                         qT_sb[p][64:128, q0 + vc:q0 + 512],
                                         start=True, stop=True)
                        ET = etp.tile([128, 1024], F32R, tag="e", name="ET")
                        if diag:
                            nc.scalar.activation(ET[:, vc:512], S12[:, vc:512],
                                                 Act.Exp)
                            nc.scalar.activation(ET[:, 512 + vc:1024],
                                                 S12[:, 512 + vc:1024], Act.Exp)
                            if t < 3:
                                for b0 in (128 * t, 512 + 128 * t):
                                    nc.gpsimd.tensor_mul(ET[:, b0:b0 + 128],
                                                         ET[:, b0:b0 + 128],
                                                         tri[:])
                            else:
                                for b0 in (256, 768):
                                    nc.gpsimd.tensor_mul(ET[:, b0:b0 + 256],
                                                         ET[:, b0:b0 + 256],
                                                         m3[:])
                        else:
                            nc.scalar.activation(ET[:], S12[:], Act.Exp)
                        cur = (ki, slice(vc, 512), ET, slice(vc, 512),
                               slice(512 + vc, 1024))
                        if prev is not None:
                            avr(prev, t == 1, False)
                        prev = cur
                    avr(prev, n_tiles == 1, True)

                    # epilogue: u = O1*R2 - lam*O2*R1 (RMS-norm cancels R1*R2)
                    # engines read at most one PSUM operand per instruction,
                    # so the row sums are staged through SBUF first
                    r2s = ebp.tile([128, 512], F32, tag="eb", name="r2s")
                    nc.vector.tensor_copy(r2s[:], R2[:])
                    c1 = ebp.tile([128, 512], F32, tag="eb", name="c1")
                    nc.vector.tensor_mul(c1[:], OT1[:], r2s[:])
                    r1s = ebp.tile([128, 512], F32, tag="eb", name="r1s")
                    nc.scalar.copy(r1s[:], R1[:])
                    c2 = ebp.tile([128, 512], F32, tag="eb", name="c2")
                    nc.vector.tensor_mul(c2[:], OT2[:], r1s[:])
                    rr = ebp.tile([128, 512], F32, tag="eb", name="rr")
                    nc.gpsimd.tensor_mul(rr[:], r1s[:], r2s[:])
                    s1 = ebp.tile([128, 512], F32, tag="eb", name="s1")
                    # SEPS/ZS: scales pre by ZS^-2 so sf = ZS * rsqrt(pre)
                    nc.gpsimd.tensor_scalar_mul(s1[:], rr[:], SEPS / ZS)
                    u = ebp.tile([128, 512], F32R, tag="eb", name="u")
                    nc.vector.scalar_tensor_tensor(
                        u[:], c2[:], nlam[:, 0:1], c1[:], Alu.mult, Alu.add)
                    t2 = ebp.tile([128, 512], F32, tag="eb", name="t2")
                    nc.gpsimd.tensor_mul(t2[:], s1[:], s1[:])
                    sq = ebp.tile([128, 512], F32R, tag="eb", name="sq")
                    nc.vector.tensor_mul(sq[:], u[:], u[:])
                    # variance column-sum on the gpsimd partition reducer:
                    # no PE matmul, no psum slot, so the next iteration's
                    # accumulators are never blocked on this epilogue
                    varp = ebp.tile([128, 512], F32, tag="eb", name="varp")
                    nc.gpsimd.partition_all_reduce(varp[:], sq[:], 128,
                                                  bass_isa.ReduceOp.add)
                    pre = ebp.tile([128, 512], F32, tag="eb", name="pre")
                    nc.vector.scalar_tensor_tensor(
                        pre[:], varp[:], 1.0 / (128.0 * ZS * ZS), t2[:],
                        Alu.mult, Alu.add)
                    lnp = ebp.tile([128, 512], F32, tag="eb", name="lnp")
                    nc.scalar.activation(lnp[:], pre[:], Act.Ln)
                    sf = ebp.tile([128, 512], F32, tag="eb", name="sf")
                    nc.scalar.activation(sf[:], lnp[:], Act.Exp, scale=-0.5)
                    z = ebp.tile([128, 512], F32, tag="eb", name="z")
                    nc.vector.tensor_mul(z[:], u[:], sf[:])
                    oh = otf_h[p // 2][:, p % 2, qsl]
                    nc.gpsimd.tensor_copy(oh, z[:])
                    nc.vector.tensor_sub(otf_l[p // 2][:, p % 2, qsl], z[:], oh)

              # ---------------- Phase C: o_proj (fp8 DoubleRow partial) ---
              with tc.tile_pool(name="outp", bufs=3) as outp, \
                 tc.tile_pool(name="psC", bufs=6, space="PSUM") as psC:
                for m in range(16):
                    osb = outp.tile([128, 2048], BF16, tag="ob", name="osb")
                    msl = slice(m * 128, m * 128 + 128)
                    for n in range(4):
                        ps = psC.tile([128, 512], F32, tag="pc", name="pc")
                        first = True
                        for pr in range(2):
                            for an, wn in (("h", "h"), ("l", "h"),
                                           ("h", "l")):
                                at = (otf_h if an == "h" else otf_l)[pr]
                                nc.tensor.matmul(
                                    ps[:],
                                    at[:, :, msl],
                                    wo_r[wn, n][:, pr, :, :],
                                    start=first, stop=(pr == 1 and
                                                       an == "h" and
                                                       wn == "l"),
                                    perf_mode=DR)
                                first = False
                        # combines alternate DVE/Act so neither ties with
                        # the phase-C matmul stream
                        if n % 2 == 0:
                            nc.vector.tensor_copy(
                                osb[:, n * 512:n * 512 + 512], ps[:])
                        else:
                            nc.scalar.copy(
                                osb[:, n * 512:n * 512 + 512], ps[:])
                        # per-n output DMA: overlaps the next n's matmuls
                        nc.sync.dma_start(
                            out=out_d[m * 128:m * 128 + 128,
                                      n * 512:n * 512 + 512],
                            in_=osb[:, n * 512:n * 512 + 512])

    nc.compile()
    return nc


def _host_tables():
    inv = ROPE_THETA ** (-np.arange(Dh, dtype=np.float64) / Dh)
    pos = np.arange(S, dtype=np.float64)
    fr = pos[:, None] * inv[None, :]              # [S, 64]
    cos = np.cos(fr).astype(np.float32)           # [S, 64]
    sin = np.sin(fr).astype(np.float32)
    d = np.arange(128) % 64
    cosT = cos[:, d].T.copy()                     # [128, S]
    sgn = np.where((np.arange(128) % 64) < 32, -1.0, 1.0).astype(np.float32)
    ssinT = (sin[:, d].T * sgn[:, None]).copy()
    tri = np.triu(np.ones((128, 128), np.float32))  # tri[k, q] = 1 if q >= k
    m3 = np.concatenate([np.zeros((128, 128), np.float32), tri], axis=1)
    ones = np.ones((128, 128), np.float32)
    return (np.ascontiguousarray(cosT), np.ascontiguousarray(ssinT), tri,
            np.ascontiguousarray(m3), ones)


def _fp8_split(a):
    import ml_dtypes
    E4 = ml_dtypes.float8_e4m3
    hi = np.ascontiguousarray(a).astype(E4)
    lo = (a - hi.astype(np.float32)).astype(E4)
    return hi, lo


def kernel(hidden_states, Wq, Wk, Wv, Wo,
           lambda_q1, lambda_k1, lambda_q2, lambda_k2, subln_weight):
    from concourse.bass_utils import run_bass_kernel_spmd

    if "nc" not in _CACHE:
        _CACHE["nc"] = _build_nc()
        _CACHE["tables"] = _host_tables()
    nc = _CACHE["nc"]
    cosT, ssinT, tri, m3, ones = _CACHE["tables"]

    f32 = np.float32
    hs = np.asarray(hidden_states, f32)
    Wq = np.asarray(Wq, f32)
    Wk = np.asarray(Wk, f32)
    Wv = np.asarray(Wv, f32)
    Wo = np.asarray(Wo, f32)
    subln = np.asarray(subln_weight, f32)

    lam1 = np.exp(np.sum(np.asarray(lambda_q1, f32) * np.asarray(lambda_k1, f32),
                         dtype=f32))
    lam2 = np.exp(np.sum(np.asarray(lambda_q2, f32) * np.asarray(lambda_k2, f32),
                         dtype=f32))
    lam_full = f32(lam1 - lam2 + LAMBDA_INIT)
    nlam_arr = np.full((128, 1), -lam_full, f32)

    scale = f32(Dh ** -0.5)
    wprime = (np.tile(subln, H) * f32(1.0 - LAMBDA_INIT)).astype(f32)  # [2048]
    WoS = Wo * wprime[None, :]

    in_maps = []
    for c in range(NCORES):
        b, r = c // TP, c % TP
        xh, xl = _fp8_split(hs[b].T * f32(XS))
        wqh, wql = _fp8_split((Wq[512 * r:512 * r + 512, :].T * f32(scale * WQS)))
        wkv = np.concatenate(
            [Wk[256 * r:256 * r + 256, :].T * f32(WKS),
             Wv[256 * r:256 * r + 256, :].T * f32(WVS)], axis=1)
        wkvh, wkvl = _fp8_split(wkv)
        woh, wol = _fp8_split(WoS[:, 512 * r:512 * r + 512].T * f32(WOS))
        in_maps.append({
            "xh": xh, "xl": xl,
            "wqh": wqh, "wql": wql, "wkvh": wkvh, "wkvl": wkvl,
            "woh": woh, "wol": wol,
            "cosT": cosT, "ssinT": ssinT, "tri": tri, "m3": m3, "ones": ones,
            "nlam": nlam_arr,
        })

    res = run_bass_kernel_spmd(nc, in_maps, core_ids=list(range(NCORES)))
    out = np.zeros((B, S, D), f32)
    ods = f32(1.0 / (ZS * WOS))
    for c in range(NCORES):
        out[c // TP] += res.results[c]["out"].astype(f32) * ods
    return out
